# revision 49
# baseline (speedup 1.0000x reference)
"""MinimalMOIRAI dense transformer on 8 Trainium2 NeuronCores.

Sharding: core c -> (batch b = c//2) x (query half = c%2). Each core holds
full K/V for its batch element and computes queries / FFN / LN / head for its
half of S. Columns are locally rolled so "my half" is always cols 0..1023 --
the SPMD program is identical on all cores; all per-core variation arrives
through input data. The residual stream h is exchanged once between layers
via a pairwise AllReduce(add) + local subtract (other = sum - mine).

Layout: residual hT kept feature-major [D, S]. RoPE applied on-chip: the raw
projection k is rotated by a fixed pairwise permutation matmul (protT) and
combined k*cos + rot(k)*sin on DVE -- one projection matmul per head instead
of the dual-projection trick (halves Wq/Wk traffic and matmul cycles).

K and Q live in even/odd head copies kTh2/qTh2 [128, 4, 2, *]: for head pair
(2c, 2c+1), copy 0 holds the even head's features at rows 0..63 with 8
variate-indicator rows at 64..71 (window [0:72]); copy 1 holds the odd
head's features at rows 64..127 with the indicator rows at 0..7 and zeros
at 8..63 (window [0:128]). The scores matmul then contracts
the rope'd features plus the one-hot variate rows together, so the per-head
variate bias delta[l,h]*[same-variate(q,k)] lands inside the scores PSUM
with no per-stripe fixups (this removes all GpSimd work from the attention
inner loop). Scores are computed transposed [k_part, q_free] in
f16; softmax uses exp without max-subtraction (|s| < 2 for this model
family); the mask enters as a per-partition bias inside the exp activation.
The softmax denominator comes free from a ones-column appended to V;
normalization is applied after PV via a small ones-matmul partition
broadcast of the reciprocal row.
"""
import os
import sys
from contextlib import ExitStack

sys.path.insert(0, "/opt/trn_rl_repo")

import numpy as np
import ml_dtypes

import concourse.bass as bass
import concourse.tile as tile
from concourse import bacc
from concourse import mybir
from concourse.bass_utils import run_bass_kernel_spmd
from concourse import bass2jax as _b2j

# NEFF disk cache keyed by BIR hash -- the program embeds no input values,
# so identical shapes reuse the compiled NEFF across calls and processes.
import hashlib
import shutil

_ORIG_CBK = _b2j.compile_bir_kernel


def _cached_compile_bir_kernel(bir_json, tmpdir, neff_name="file.neff"):
    h = hashlib.sha256(bir_json).hexdigest()[:24]
    cache_dir = "/tmp/bass_neff_cache"
    cpath = os.path.join(cache_dir, f"{h}.neff")
    dst = os.path.join(tmpdir, neff_name)
    if os.path.exists(cpath):
        shutil.copy(cpath, dst)
        return dst
    out = _ORIG_CBK(bir_json, tmpdir, neff_name=neff_name)
    os.makedirs(cache_dir, exist_ok=True)
    tmp_c = cpath + ".tmp"
    shutil.copy(out, tmp_c)
    os.replace(tmp_c, cpath)
    return out


_b2j.compile_bir_kernel = _cached_compile_bir_kernel

dt = mybir.dt
AF = mybir.ActivationFunctionType
ALU = mybir.AluOpType

B, S, P, D, H, L, DFF = 4, 2048, 32, 512, 8, 2, 2048
HD = D // H
NV = 8  # variates
NQ = S // 2  # queries per core
SCALE = 1.0 / np.sqrt(HD)
DC = D // 128  # 4 feature chunks
FC = DFF // 128  # 16 dff chunks
NKC = S // 128  # 16 key chunks
RG = [[0, 1], [2, 3], [4, 5], [6, 7]]

F32 = dt.float32
F16 = dt.float16


def build_program() -> bass.Bass:
    nc = bacc.Bacc(None, target_bir_lowering=False, num_devices=8)

    # ---- I/O declarations (per-core data) ----
    xT_d = nc.declare_dram_parameter("xT", [128, S], F16, isOutput=False)
    wpe_d = nc.declare_dram_parameter("wpe", [128, D], F16, isOutput=False)
    bpe_d = nc.declare_dram_parameter("bpe", [128, DC], F32, isOutput=False)
    rcos_d = nc.declare_dram_parameter("rcos", [128, S], F16, isOutput=False)
    rsin_d = nc.declare_dram_parameter("rsin", [128, S], F16, isOutput=False)
    maskb_d = nc.declare_dram_parameter("maskb", [128, NKC], F32, isOutput=False)
    protT_d = nc.declare_dram_parameter("protT", [128, 128], F16, isOutput=False)
    kvar_d = nc.declare_dram_parameter("kvar", [8, S], F16, isOutput=False)
    qind_d = nc.declare_dram_parameter("qind", [128, NQ], F16, isOutput=False)
    dvals_d = nc.declare_dram_parameter("dvals", [128, L * H], F32, isOutput=False)
    # weights pre-chunked host-side for contiguous per-partition DMA runs
    wq_d = nc.declare_dram_parameter("wq", [L, DC, 128, DC, 128], F16, isOutput=False)
    wk_d = nc.declare_dram_parameter("wk", [L, DC, 128, DC, 128], F16, isOutput=False)
    wv_d = nc.declare_dram_parameter("wv", [L, D, H * 65], F16, isOutput=False)
    bv_d = nc.declare_dram_parameter("bv", [L, 1, H * 65], F32, isOutput=False)
    wo_d = nc.declare_dram_parameter("wo", [L, DC, 64, 8, 128], F16, isOutput=False)
    bo_d = nc.declare_dram_parameter("bo", [L, 128, DC], F32, isOutput=False)
    w1_d = nc.declare_dram_parameter("w1", [L, FC, 128, DC, 128], F16, isOutput=False)
    b1_d = nc.declare_dram_parameter("b1", [L, 128, FC], F32, isOutput=False)
    w2_d = nc.declare_dram_parameter("w2", [L, DFF, D], F16, isOutput=False)
    b2_d = nc.declare_dram_parameter("b2", [L, 128, DC], F32, isOutput=False)
    g1_d = nc.declare_dram_parameter("g1c", [L, 128, DC], F32, isOutput=False)
    be1_d = nc.declare_dram_parameter("be1c", [L, 128, DC], F32, isOutput=False)
    g2_d = nc.declare_dram_parameter("g2c", [L, 128, DC], F32, isOutput=False)
    be2_d = nc.declare_dram_parameter("be2c", [L, 128, DC], F32, isOutput=False)
    wh_d = nc.declare_dram_parameter("wh", [128, DC, P], F16, isOutput=False)
    bh_d = nc.declare_dram_parameter("bh", [1, P], F32, isOutput=False)
    out_d = nc.declare_dram_parameter("outp", [NQ, P], F32, isOutput=True)

    with tile.TileContext(nc) as tc, \
            nc.allow_low_precision(reason="fp16 matmul operands, f32 psum accumulation"):
        _stack = ExitStack()
        sb = _stack.enter_context(tc.tile_pool(name="sb", bufs=1))
        stream = _stack.enter_context(tc.tile_pool(name="stream", bufs=1))
        dram = _stack.enter_context(tc.tile_pool(name="dram", bufs=1, space="DRAM"))
        cc_in = dram.tile([128, DC, NQ], F32, name="cc_in")
        cc_out = dram.tile([128, DC, NQ], F32, name="cc_out")

        # ---- persistent tiles ----
        ones72 = sb.tile([72, 64], F16)
        nc.vector.memset(ones72, 1.0)
        ones1 = sb.tile([128, 1], F16)
        nc.vector.memset(ones1, 1.0)
        ones8w = sb.tile([8, 128], F16)
        nc.vector.memset(ones8w, 1.0)
        eps1 = sb.tile([1, 1], F32)
        nc.vector.memset(eps1, 1e-5)

        # embed inputs first -- they unblock the first matmuls
        xT = sb.tile([128, S], F16, tag="slot8")
        nc.sync.dma_start(out=xT, in_=xT_d[:])
        wpe = sb.tile([128, D], F16)
        nc.scalar.dma_start(out=wpe, in_=wpe_d[:])
        bpe = sb.tile([128, DC], F32)
        nc.scalar.dma_start(out=bpe, in_=bpe_d[:])

        rcos = sb.tile([128, S], F16)
        nc.sync.dma_start(out=rcos, in_=rcos_d[:])
        rsin = sb.tile([128, S], F16)
        nc.scalar.dma_start(out=rsin, in_=rsin_d[:])
        maskb = sb.tile([128, NKC], F32)
        nc.scalar.dma_start(out=maskb, in_=maskb_d[:])
        bh_b = sb.tile([128, P], F32)
        nc.gpsimd.dma_start(out=bh_b, in_=bh_d[:].to_broadcast((128, P)))
        protT = sb.tile([128, 128], F16)
        nc.sync.dma_start(out=protT, in_=protT_d[:])
        qTv = sb.tile([128, NQ], F16)
        nc.scalar.dma_start(out=qTv, in_=qind_d[:])
        dvalsb = sb.tile([128, L * H], F32)
        nc.scalar.dma_start(out=dvalsb, in_=dvals_d[:])

        # per-headpair K/Q copies with 8 variate-indicator contraction rows:
        # copy 0 (even head 2c): feats rows 0..63, indicators rows 64..71,
        #   contraction window [0:72]
        # copy 1 (odd head 2c+1): indicators rows 0..7, zeros rows 8..63,
        #   feats rows 64..127, contraction window [0:128]
        kTh2 = sb.tile([128, DC, 2, S], F16, name="kTh2")
        qTh2 = sb.tile([128, DC, 2, NQ], F16, name="qTh2")
        for c in range(DC):
            nc.vector.memset(kTh2[0:64, c, 1, :], 0.0)
            nc.sync.dma_start(out=kTh2[64:72, c, 0, :], in_=kvar_d[:])
            nc.sync.dma_start(out=kTh2[0:8, c, 1, :], in_=kvar_d[:])

        # ---- embed: hT[f, s] = (x @ Wpe + bpe)^T ----
        hT = sb.tile([128, DC, S], F16, tag="hT", name="hT0")
        with tc.tile_pool(name="ps_embed", bufs=4, space="PSUM") as ps_e:
            for m in range(DC):
                for n in range(4):
                    pe = ps_e.tile([128, 512], F32, tag="pe")
                    nc.tensor.matmul(
                        pe, lhsT=wpe[:, m * 128:(m + 1) * 128],
                        rhs=xT[:, n * 512:(n + 1) * 512],
                        start=True, stop=True)
                    nc.scalar.activation(
                        out=hT[:, m, n * 512:(n + 1) * 512], in_=pe,
                        func=AF.Identity, bias=bpe[:, m:m + 1], scale=1.0)

        h_fin = None
        for l in range(L):
            # variate rows of qTh2: delta[l, h] * indicator
            for c in range(DC):
                he, ho = l * H + 2 * c, l * H + 2 * c + 1
                nc.vector.tensor_scalar(
                    out=qTh2[64:72, c, 0, :], in0=qTv[64:72, :],
                    scalar1=dvalsb[64:72, he:he + 1], scalar2=None, op0=ALU.mult)
                nc.vector.tensor_scalar(
                    out=qTh2[0:64, c, 1, :], in0=qTv[0:64, :],
                    scalar1=dvalsb[0:64, ho:ho + 1], scalar2=None, op0=ALU.mult)

            # ======== Phase A: K/Q projections + on-chip rope ========
            # For l>0 the other-half columns of hT arrive via the AllReduce;
            # emit my-half work (q, k cols 0:NQ, v rows 0:NQ) first so the
            # tensor engine computes through the collective wait.
            vA = sb.tile([128, NKC, H * 65], F16, tag="slotB", name=f"v{l}")
            wv = sb.tile([128, DC, H * 65], F16, tag="wv", name=f"wv{l}")
            nc.scalar.dma_start(out=wv, in_=wv_d[l].rearrange("(kc p) m -> p kc m", p=128))
            bv_b = sb.tile([128, H * 65], F32, tag="bvb", name=f"bv{l}")
            nc.gpsimd.dma_start(out=bv_b, in_=bv_d[l].to_broadcast((128, H * 65)))

            with tc.tile_pool(name=f"ps_proj{l}", bufs=2, space="PSUM") as ps_p, \
                 tc.tile_pool(name=f"ps_rot{l}", bufs=2, space="PSUM") as ps_r, \
                 tc.tile_pool(name=f"ps_v{l}", bufs=2, space="PSUM") as ps_v:

                def _kq(m, wk_m, wq_m, ns, do_q):
                    kraws = []
                    for n in ns:
                        cols = slice(n * 512, (n + 1) * 512)
                        psp = ps_p.tile([128, 512], F32, tag="pp", name="psp")
                        for kc in range(DC):
                            nc.tensor.matmul(
                                psp, lhsT=wk_m[:, kc],
                                rhs=hT[:, kc, cols],
                                start=(kc == 0), stop=(kc == DC - 1))
                        kraw = stream.tile([128, 512], F16, tag="kraw", bufs=6, name="kraw")
                        nc.scalar.activation(out=kraw, in_=psp, func=AF.Identity)
                        kraws.append(kraw)
                        if do_q and n < 2:
                            psq = ps_p.tile([128, 512], F32, tag="pp", name="psq")
                            for kc in range(DC):
                                nc.tensor.matmul(
                                    psq, lhsT=wq_m[:, kc],
                                    rhs=hT[:, kc, cols],
                                    start=(kc == 0), stop=(kc == DC - 1))
                            qraw = stream.tile([128, 512], F16, tag="kraw", bufs=6, name="qraw")
                            nc.scalar.activation(out=qraw, in_=psq, func=AF.Identity)
                            kraws.append(qraw)
                    idx = 0
                    for n in ns:
                        cols = slice(n * 512, (n + 1) * 512)
                        kraw = kraws[idx]; idx += 1
                        rot = ps_r.tile([128, 512], F32, tag="rr", name="rot")
                        nc.tensor.matmul(rot, lhsT=protT, rhs=kraw,
                                         start=True, stop=True)
                        t1 = stream.tile([128, 512], F32, tag="rt", bufs=4, name="t1")
                        t2 = stream.tile([128, 512], F32, tag="rt", bufs=4, name="t2")
                        nc.vector.tensor_tensor(out=t1, in0=kraw, in1=rcos[:, cols], op=ALU.mult)
                        nc.vector.tensor_tensor(out=t2, in0=rot, in1=rsin[:, cols], op=ALU.mult)
                        nc.vector.tensor_tensor(out=kTh2[0:64, m, 0, cols],
                                                in0=t1[0:64], in1=t2[0:64], op=ALU.add)
                        nc.vector.tensor_tensor(out=kTh2[64:128, m, 1, cols],
                                                in0=t1[64:128], in1=t2[64:128], op=ALU.add)
                        if do_q and n < 2:
                            qraw = kraws[idx]; idx += 1
                            rotq = ps_r.tile([128, 512], F32, tag="rr", name="rotq")
                            nc.tensor.matmul(rotq, lhsT=protT, rhs=qraw,
                                             start=True, stop=True)
                            t3 = stream.tile([128, 512], F32, tag="rt", bufs=4, name="t3")
                            t4 = stream.tile([128, 512], F32, tag="rt", bufs=4, name="t4")
                            nc.vector.tensor_tensor(out=t3, in0=qraw, in1=rcos[:, cols], op=ALU.mult)
                            nc.vector.tensor_tensor(out=t4, in0=rotq, in1=rsin[:, cols], op=ALU.mult)
                            nc.vector.tensor_tensor(out=qTh2[0:64, m, 0, cols],
                                                    in0=t3[0:64], in1=t4[0:64], op=ALU.add)
                            nc.vector.tensor_tensor(out=qTh2[64:128, m, 1, cols],
                                                    in0=t3[64:128], in1=t4[64:128], op=ALU.add)

                def _vproj(mts):
                    for mt in mts:
                        rows = slice(mt * 128, (mt + 1) * 128)
                        pv = ps_v.tile([128, H * 65], F32, tag="pv")
                        for kc in range(DC):
                            nc.tensor.matmul(
                                pv[:, 0:512], lhsT=hT[:, kc, rows],
                                rhs=wv[:, kc, 0:512],
                                start=(kc == 0), stop=(kc == DC - 1))
                        for kc in range(DC):
                            nc.tensor.matmul(
                                pv[:, 512:520], lhsT=hT[:, kc, rows],
                                rhs=wv[:, kc, 512:520],
                                start=(kc == 0), stop=(kc == DC - 1))
                        nc.vector.tensor_tensor(out=vA[:, mt], in0=pv, in1=bv_b, op=ALU.add)

                wks, wqs = [], []
                for m in range(DC):
                    wk_m = stream.tile([128, DC, 128], F16, tag="wkq", bufs=8, name=f"wk{l}{m}")
                    nc.sync.dma_start(out=wk_m, in_=wk_d[l, m])
                    wq_m = stream.tile([128, DC, 128], F16, tag="wkq", bufs=8, name=f"wq{l}{m}")
                    nc.scalar.dma_start(out=wq_m, in_=wq_d[l, m])
                    wks.append(wk_m); wqs.append(wq_m)
                if l == 0:
                    for m in range(DC):
                        _kq(m, wks[m], wqs[m], [0, 1, 2, 3], True)
                    _vproj(range(NKC))
                else:
                    # my-half first (independent of the collective)
                    for m in range(DC):
                        _kq(m, wks[m], wqs[m], [0, 1], True)
                    _vproj(range(NKC // 2))
                    for m in range(DC):
                        _kq(m, wks[m], wqs[m], [2, 3], False)
                    _vproj(range(NKC // 2, NKC))

            # ======== Phase B: attention ========
            # prefetch Phase C weights while the DMA queues are quiet
            bo_c = sb.tile([128, DC], F32, tag="boc", name=f"bo{l}")
            nc.scalar.dma_start(out=bo_c, in_=bo_d[l])
            wos = []
            for m in range(DC):
                wo_m = stream.tile([64, 8, 128], F16, tag="wo", bufs=4, name=f"wo{l}{m}")
                nc.sync.dma_start(out=wo_m, in_=wo_d[l, m])
                wos.append(wo_m)

            # attnraw rows 0..63: unnormalized PV output (normalized in place
            # one head behind); row 64: softmax denominator. The cheap DVE
            # copy frees o_ps immediately so the next head's PV can start.
            attnraw = sb.tile([65, H, NQ], F16, tag="attn", name=f"attn{l}")
            rec = stream.tile([72, NQ], F16, tag="rec", bufs=1, name=f"rec{l}")
            nc.vector.memset(rec[64:72, :], 0.0)

            def _attn_norm(h, pool):
                nc.vector.reciprocal(out=rec[64:65, :], in_=attnraw[64:65, h, :])
                rb_ps = pool.tile([64, NQ], F32, tag="sps", name="rb_ps")
                for qn in range(2):
                    nc.tensor.matmul(
                        rb_ps[:, qn * 512:(qn + 1) * 512],
                        lhsT=ones72[64:72, :],
                        rhs=rec[64:72, qn * 512:(qn + 1) * 512],
                        start=True, stop=True)
                nc.vector.tensor_tensor(out=attnraw[0:64, h, :],
                                        in0=attnraw[0:64, h, :],
                                        in1=rb_ps, op=ALU.mult)

            # two heads in flight: keeps tensor dense through the exp stream
            with tc.tile_pool(name=f"ps_sc{l}", bufs=2, space="PSUM") as ps_s, \
                 tc.tile_pool(name=f"ps_o{l}", bufs=2, space="PSUM") as ps_o:
                for hp in range(H // 2):
                    hA, hB = 2 * hp, 2 * hp + 1
                    c = hp
                    o_psA = ps_o.tile([128, NQ], F32, tag="ops", name="oA")
                    o_psB = ps_o.tile([128, NQ], F32, tag="ops", name="oB")
                    for kc in range(NKC):
                        for hh, o_ps in ((hA, o_psA), (hB, o_psB)):
                            eo = hh % 2
                            r0, rn = (0, 72) if eo == 0 else (0, 128)
                            s_ps = ps_s.tile([128, NQ], F32, tag="sps")
                            for qn in range(2):
                                nc.tensor.matmul(
                                    s_ps[:, qn * 512:(qn + 1) * 512],
                                    lhsT=kTh2[r0:r0 + rn, c, eo, kc * 128:(kc + 1) * 128],
                                    rhs=qTh2[r0:r0 + rn, c, eo, qn * 512:(qn + 1) * 512],
                                    start=True, stop=True)
                            eT = stream.tile([128, NQ], F16, tag="eT", bufs=4, name="eT")
                            nc.scalar.activation(out=eT, in_=s_ps, func=AF.Exp,
                                                 bias=maskb[:, kc:kc + 1], scale=1.0)
                            for qn in range(2):
                                nc.tensor.matmul(
                                    o_ps[0:65, qn * 512:(qn + 1) * 512],
                                    lhsT=vA[:, kc, hh * 65:(hh + 1) * 65],
                                    rhs=eT[:, qn * 512:(qn + 1) * 512],
                                    start=(kc == 0), stop=(kc == NKC - 1),
                                    skip_group_check=True)
                        # normalize the previous pair mid-loop (recycles the
                        # o_ps pool buffers for the rb broadcasts)
                        if hp > 0:
                            if kc == 6:
                                _attn_norm(hA - 2, ps_s)
                            elif kc == 12:
                                _attn_norm(hB - 2, ps_s)
                    nc.vector.tensor_copy(out=attnraw[:, hA, :], in_=o_psA[0:65, :])
                    nc.vector.tensor_copy(out=attnraw[:, hB, :], in_=o_psB[0:65, :])

            # ======== Phase C: O-projection + residual + LN1 ========
            # head-chunk-outer accumulation: heads 6/7's trailing
            # normalizations hide under the c<6 matmuls
            hraw = sb.tile([128, DC, NQ], F16, tag="slotA", name=f"hraw{l}")
            with tc.tile_pool(name=f"ps_oproj{l}", bufs=4, space="PSUM") as ps_op, \
                 tc.tile_pool(name=f"ps_orb{l}", bufs=1, space="PSUM") as ps_orb:
                for n2 in range(2):
                    cols = slice(n2 * 512, (n2 + 1) * 512)
                    pos = [ps_op.tile([128, 512], F32, tag="po",
                                      name=f"po_{l}_{n2}_{m}") for m in range(DC)]
                    for c in range(8):
                        if n2 == 0 and c == 6:
                            _attn_norm(H - 2, ps_orb)
                        elif n2 == 0 and c == 7:
                            _attn_norm(H - 1, ps_orb)
                        for m in range(DC):
                            nc.tensor.matmul(
                                pos[m], lhsT=wos[m][:, c],
                                rhs=attnraw[0:64, c, cols],
                                start=(c == 0), stop=(c == 7),
                                skip_group_check=True)
                    for m in range(DC):
                        to = stream.tile([128, 512], F32, tag="rt", bufs=4, name="to")
                        nc.scalar.activation(out=to, in_=pos[m],
                                             func=AF.Identity,
                                             bias=bo_c[:, m:m + 1], scale=1.0)
                        nc.vector.tensor_tensor(out=hraw[:, m, cols], in0=to,
                                                in1=hT[:, m, cols], op=ALU.add)

            g1c = sb.tile([128, DC], F32, tag="g1", name=f"g1{l}")
            nc.scalar.dma_start(out=g1c, in_=g1_d[l])
            be1c = sb.tile([128, DC], F32, tag="be1", name=f"be1{l}")
            nc.scalar.dma_start(out=be1c, in_=be1_d[l])
            h1 = sb.tile([128, DC, NQ], F16, tag="slotB", name=f"h1_{l}")
            _layernorm(nc, tc, stream, hraw, h1, g1c, be1c, ones1, ones8w, eps1, f"ln1_{l}")

            # ======== Phase D: FFN + residual + LN2 ========
            b1c = sb.tile([128, FC], F32, tag="b1c", name=f"b1{l}")
            nc.scalar.dma_start(out=b1c, in_=b1_d[l])
            b2c = sb.tile([128, DC], F32, tag="b2c", name=f"b2{l}")
            nc.scalar.dma_start(out=b2c, in_=b2_d[l])
            hraw2 = sb.tile([128, DC, NQ], F16, tag="slotA", name=f"hraw2_{l}")
            with tc.tile_pool(name=f"ps_ffn{l}", bufs=2, space="PSUM") as ps_f1, \
                 tc.tile_pool(name=f"ps_ffn2{l}", bufs=1, space="PSUM") as ps_f2:
                for n2 in range(2):
                    cols = slice(n2 * 512, (n2 + 1) * 512)
                    p2s = [ps_f2.tile([128, 512], F32, tag=f"p2_{m}", name=f"p2_{l}_{n2}_{m}") for m in range(DC)]
                    for dc in range(FC):
                        w1_t = stream.tile([128, DC, 128], F16, tag="wst", bufs=4, name="w1t")
                        nc.sync.dma_start(out=w1_t, in_=w1_d[l, dc])
                        p1 = ps_f1.tile([128, 512], F32, tag="p1")
                        for kc in range(DC):
                            nc.tensor.matmul(
                                p1, lhsT=w1_t[:, kc], rhs=h1[:, kc, cols],
                                start=(kc == 0), stop=(kc == DC - 1))
                        fT = stream.tile([128, 512], F16, tag="fT", bufs=2, name="fT")
                        nc.scalar.activation(out=fT, in_=p1, func=AF.Gelu,
                                             bias=b1c[:, dc:dc + 1], scale=1.0)
                        w2_t = stream.tile([128, D], F16, tag="w2t", bufs=2, name="w2t")
                        nc.scalar.dma_start(out=w2_t, in_=w2_d[l].rearrange(
                            "(dc p) m -> p dc m", p=128)[:, dc])
                        for m in range(DC):
                            nc.tensor.matmul(
                                p2s[m], lhsT=w2_t[:, m * 128:(m + 1) * 128],
                                rhs=fT,
                                start=(dc == 0), stop=(dc == FC - 1),
                                skip_group_check=True)
                    for m in range(DC):
                        tf = stream.tile([128, 512], F32, tag="rt", bufs=4, name="tf")
                        nc.scalar.activation(out=tf, in_=p2s[m], func=AF.Identity,
                                             bias=b2c[:, m:m + 1], scale=1.0)
                        nc.vector.tensor_tensor(out=hraw2[:, m, cols], in0=tf,
                                                in1=h1[:, m, cols], op=ALU.add)

            g2c = sb.tile([128, DC], F32, tag="g2", name=f"g2{l}")
            nc.scalar.dma_start(out=g2c, in_=g2_d[l])
            be2c = sb.tile([128, DC], F32, tag="be2", name=f"be2{l}")
            nc.scalar.dma_start(out=be2c, in_=be2_d[l])

            if l == 0:
                hT2 = sb.tile([128, DC, S], F16, tag="hT", name="hT1")
                h2view = hT2[:, :, 0:NQ]
                _layernorm(nc, tc, stream, hraw2, h2view, g2c, be2c, ones1, ones8w, eps1, f"ln2_{l}")
                # exchange: other = AllReduce(mine) - mine (f16->f32 cast: SWDGE)
                nc.gpsimd.dma_start(out=cc_in, in_=hT2[:, :, 0:NQ])
                if os.environ.get("KBENCH_SKIP_CC"):
                    nc.sync.dma_start(out=cc_out, in_=cc_in)
                else:
                    nc.gpsimd.collective_compute(
                        "AllReduce", ALU.add, replica_groups=RG,
                        ins=[cc_in.opt()], outs=[cc_out.opt()])
                # split halves so the deferred K-proj (cols 1024:1536 first)
                # unblocks as early as possible
                for half in range(2):
                    hs = slice(half * 512, (half + 1) * 512)
                    for c in range(DC):
                        Rc = stream.tile([128, 512], F32, tag="Rc", bufs=2, name="Rc")
                        nc.sync.dma_start(out=Rc, in_=cc_out[:, c, hs])
                        nc.vector.tensor_tensor(out=hT2[:, c, NQ + half * 512:NQ + (half + 1) * 512],
                                                in0=Rc, in1=hT2[:, c, half * 512:(half + 1) * 512],
                                                op=ALU.subtract)
                hT = hT2
            else:
                h_fin = sb.tile([128, DC, NQ], F16, tag="slotB", name="hfin")
                _layernorm(nc, tc, stream, hraw2, h_fin, g2c, be2c, ones1, ones8w, eps1, f"ln2_{l}")

        # ======== head ========
        wh = sb.tile([128, DC, P], F16)
        nc.sync.dma_start(out=wh, in_=wh_d[:])
        out_sb = sb.tile([128, 8, P], F32)
        with tc.tile_pool(name="ps_head", bufs=4, space="PSUM") as ps_h:
            for sc in range(8):
                ph = ps_h.tile([128, P], F32, tag="ph")
                for kc in range(DC):
                    nc.tensor.matmul(
                        ph, lhsT=h_fin[:, kc, sc * 128:(sc + 1) * 128],
                        rhs=wh[:, kc],
                        start=(kc == 0), stop=(kc == DC - 1))
                nc.vector.tensor_tensor(out=out_sb[:, sc], in0=ph, in1=bh_b, op=ALU.add)
        nc.sync.dma_start(out=out_d[:].rearrange("(sc p) n -> p sc n", p=128),
                          in_=out_sb)

        _stack.close()
    nc.finalize()
    return nc


def _layernorm(nc, tc, stream, src, dst, g_c, be_c, ones1, ones8w, eps1, uname):
    """dst = LN(src) * g + be, feature-major [128, DC, NQ] tiles."""
    with tc.tile_pool(name=f"ps_st_{uname}", bufs=1, space="PSUM") as ps_st, \
         tc.tile_pool(name=f"ps_bc_{uname}", bufs=1, space="PSUM") as ps_bc:
        s1 = ps_st.tile([1, NQ], F32, tag="s1")
        s2 = ps_st.tile([1, NQ], F32, tag="s2")
        for c in range(DC):
            for n2 in range(2):
                cols = slice(n2 * 512, (n2 + 1) * 512)
                sq = stream.tile([128, 512], F16, tag="sq", bufs=2, name="sq")
                nc.vector.tensor_tensor(out=sq, in0=src[:, c, cols],
                                        in1=src[:, c, cols], op=ALU.mult)
                nc.tensor.matmul(s1[:, cols], lhsT=ones1,
                                 rhs=src[:, c, cols],
                                 start=(c == 0), stop=(c == DC - 1),
                                 skip_group_check=True)
                nc.tensor.matmul(s2[:, cols], lhsT=ones1, rhs=sq,
                                 start=(c == 0), stop=(c == DC - 1),
                                 skip_group_check=True)
        arow = stream.tile([8, NQ], F16, tag="arow", name="arow")
        mrow = stream.tile([8, NQ], F16, tag="mrow", name="mrow")
        nc.vector.memset(arow, 0.0)
        nc.vector.memset(mrow, 0.0)
        ab_ps = ps_bc.tile([128, NQ], F32, tag="abp")
        mb_ps = ps_bc.tile([128, NQ], F32, tag="mbp")
        ab = stream.tile([128, NQ], F16, tag="ab", name="ab")
        mb = stream.tile([128, NQ], F16, tag="mb", name="mb")
        # column halves pipeline: half 1's reciprocal overlaps half 0's
        # broadcast + normalize
        for qn in range(2):
            cols = slice(qn * 512, (qn + 1) * 512)
            var = stream.tile([1, 512], F16, tag="var", bufs=2, name="var")
            mean = mrow[0:1, cols]
            nc.vector.tensor_scalar(out=mean, in0=s1[0:1, cols], scalar1=1.0 / D,
                                    scalar2=None, op0=ALU.mult)
            # var = s2/D - mean^2 ; compute m2 into var first
            nc.vector.tensor_tensor(out=var, in0=mean, in1=mean, op=ALU.mult)
            nc.vector.tensor_scalar(out=var, in0=var, scalar1=-1.0, scalar2=None,
                                    op0=ALU.mult)
            nc.vector.scalar_tensor_tensor(
                out=var, in0=s2[0:1, cols], scalar=1.0 / D, in1=var,
                op0=ALU.mult, op1=ALU.add)
            nc.scalar.activation(out=var, in_=var, func=AF.Sqrt, bias=eps1, scale=1.0)
            nc.vector.reciprocal(out=arow[0:1, cols], in_=var)
            nc.vector.tensor_tensor(out=mrow[0:1, cols], in0=mrow[0:1, cols],
                                    in1=arow[0:1, cols], op=ALU.mult)
            nc.vector.tensor_scalar(out=mrow[0:1, cols], in0=mrow[0:1, cols],
                                    scalar1=-1.0, scalar2=None, op0=ALU.mult)
            nc.tensor.matmul(ab_ps[:, cols], lhsT=ones8w, rhs=arow[:, cols],
                             start=True, stop=True)
            nc.tensor.matmul(mb_ps[:, cols], lhsT=ones8w, rhs=mrow[:, cols],
                             start=True, stop=True)
            nc.vector.tensor_copy(out=ab[:, cols], in_=ab_ps[:, cols])
            nc.vector.tensor_copy(out=mb[:, cols], in_=mb_ps[:, cols])
            for c in range(DC):
                t = stream.tile([128, 512], F16, tag="lnt", bufs=2, name="lnt")
                nc.vector.tensor_tensor(out=t, in0=src[:, c, cols], in1=ab[:, cols],
                                        op=ALU.mult)
                nc.vector.tensor_tensor(out=t, in0=t, in1=mb[:, cols], op=ALU.add)
                nc.vector.tensor_scalar(out=dst[:, c, cols], in0=t,
                                        scalar1=g_c[:, c:c + 1],
                                        scalar2=be_c[:, c:c + 1], op0=ALU.mult,
                                        op1=ALU.add)


# ---------------- host side ----------------

_NC_CACHE = {}


def _get_program():
    if "nc" not in _NC_CACHE:
        _NC_CACHE["nc"] = build_program()
    return _NC_CACHE["nc"]


def _rope_tables():
    inv = 1.0 / (10000.0 ** (np.arange(0, HD, 2, dtype=np.float32) / HD))
    freqs = np.outer(np.arange(S, dtype=np.float32), inv)
    emb = np.concatenate([freqs, freqs], axis=-1)
    cos, sin = np.cos(emb), np.sin(emb)
    ch, sh = cos[:, ::2], sin[:, ::2]
    cosA = np.empty((S, HD), np.float32)
    sinB = np.empty((S, HD), np.float32)
    cosA[:, 0::2] = ch
    cosA[:, 1::2] = ch
    sinB[:, 0::2] = sh
    sinB[:, 1::2] = sh
    return cosA, sinB


def _protT_chunk():
    # rot(x)[2i] = -x[2i+1]; rot(x)[2i+1] = x[2i]  (within a 128 chunk; HD=64
    # pairs never straddle a chunk boundary). lhsT = Pf^T for out = Pf @ x.
    Pm = np.zeros((128, 128), np.float32)
    for i in range(64):
        Pm[2 * i, 2 * i + 1] = -1.0
        Pm[2 * i + 1, 2 * i] = 1.0
    return np.ascontiguousarray(Pm.T).astype(np.float16)


def _col_chunks(v):
    """[L?, X*128] -> [?, 128, X] per-partition chunk layout."""
    if v.ndim == 1:
        return np.ascontiguousarray(v.reshape(-1, 128).T.astype(np.float32))
    return np.ascontiguousarray(
        np.stack([v[i].reshape(-1, 128).T for i in range(v.shape[0])]).astype(np.float32))


def _chunk_w(w):
    """[L, K, M] -> [L, MC, 128(K sub), KC, 128(mi)] contiguous DMA runs.

    Slice [l, m] yields the lhsT tile [128, KC, 128]: partition p =
    contraction row kc*128+p, free = output features m*128+mi.
    """
    Lw, K, M = w.shape
    KC, MC = K // 128, M // 128
    arr = w.reshape(Lw, KC, 128, MC, 128)           # [l, kc, p, mo, mi]
    arr = arr.transpose(0, 3, 2, 1, 4)              # [l, mo, p, kc, mi]
    return np.ascontiguousarray(arr).astype(np.float16)


def build_in_maps(inputs):
    inp = {k: np.asarray(v) for k, v in inputs.items()}
    assert np.abs(inp["bq"]).max() == 0 and np.abs(inp["bk"]).max() == 0, \
        "nonzero q/k biases not supported by this kernel build"

    cosA, sinB = _rope_tables()

    Wq = inp["Wq"].astype(np.float32) * SCALE
    Wk = inp["Wk"].astype(np.float32)

    Wv_aug = np.zeros((L, D, H * 65), np.float32)
    bv_aug = np.zeros((L, 1, H * 65), np.float32)
    for h in range(H):
        Wv_aug[:, :, h * 65:h * 65 + 64] = inp["Wv"][:, :, h * 64:(h + 1) * 64]
        bv_aug[:, 0, h * 65:h * 65 + 64] = inp["bv"][:, h * 64:(h + 1) * 64]
        bv_aug[:, 0, h * 65 + 64] = 1.0

    # Wo rows (attn features) in 64-blocks, m-chunk major: [L, DC, 64, 8, 128]
    wo_in = inp["Wo"].reshape(L, 8, 64, DC, 128)     # [l, c, f, mo, mi]
    Wo_arr = np.ascontiguousarray(
        wo_in.transpose(0, 3, 2, 1, 4)).astype(np.float16)  # [l, mo, f, c, mi]

    delta = (inp["u_same"] - inp["u_cross"]).astype(np.float32)  # [L, H]
    dvals = np.ascontiguousarray(
        np.broadcast_to(delta.reshape(1, L * H), (128, L * H))).astype(np.float32)

    wh_arr = np.ascontiguousarray(
        inp["Wh"].reshape(DC, 128, P).transpose(1, 0, 2)).astype(np.float16)

    common = dict(
        wq=_chunk_w(Wq), wk=_chunk_w(Wk),
        wv=Wv_aug.astype(np.float16), bv=bv_aug,
        wo=Wo_arr, bo=_col_chunks(inp["bo"]),
        w1=_chunk_w(inp["W1"].astype(np.float32)), b1=_col_chunks(inp["b1f"]),
        w2=inp["W2"].astype(np.float16), b2=_col_chunks(inp["b2f"]),
        g1c=_col_chunks(inp["g1"]), be1c=_col_chunks(inp["be1"]),
        g2c=_col_chunks(inp["g2"]), be2c=_col_chunks(inp["be2"]),
        wh=wh_arr, bh=inp["bh"].reshape(1, P).astype(np.float32),
        wpe=np.pad(inp["W_pe"].astype(np.float32), ((0, 128 - P), (0, 0))).astype(np.float16),
        bpe=_col_chunks(inp["b_pe"]),
        protT=_protT_chunk(), dvals=dvals,
    )

    in_maps = []
    for core in range(8):
        b, half = core // 2, core % 2
        q0 = half * NQ
        perm = (np.arange(S) + q0) % S

        vids = inp["variate_ids"][b][perm]
        # validate the kc-aligned block structure the kernel assumes
        vb = vids.reshape(NKC, 128)
        assert (vb == vb[:, :1]).all(), "variate blocks must be 128-aligned"

        kvar = np.zeros((8, S), np.float16)
        for v in range(NV):
            kvar[v] = (vids == v).astype(np.float16)
        # q-side indicators replicated at rows 64..71 (even-head copies) and
        # 0..7 (odd-head copies)
        qind = np.zeros((128, NQ), np.float16)
        for v in range(NV):
            row = (vids[:NQ] == v).astype(np.float16)
            qind[64 + v] = row
            qind[v] = row

        mask_add = (1.0 - inp["mask"][b][perm].astype(np.float32)) * -1e9
        maskb = np.ascontiguousarray(mask_add.reshape(NKC, 128).T)

        xT = np.zeros((128, S), np.float16)
        xT[:P] = inp["x"][b][perm].T.astype(np.float16)

        rc = np.ascontiguousarray(np.tile(cosA[perm].T, (2, 1))).astype(np.float16)
        rs = np.ascontiguousarray(np.tile(sinB[perm].T, (2, 1))).astype(np.float16)

        m = dict(common)
        m.update(xT=xT, rcos=rc, rsin=rs, maskb=maskb, kvar=kvar, qind=qind)
        in_maps.append(m)
    return in_maps


def kernel(_trace=False, **inputs):
    in_maps = build_in_maps(inputs)
    nc = _get_program()
    res = run_bass_kernel_spmd(nc, in_maps, list(range(8)), trace=_trace)
    out = np.zeros((B, S, P), np.float32)
    for core in range(8):
        b, half = core // 2, core % 2
        out[b, half * NQ:(half + 1) * NQ] = res.results[core]["outp"]
    if _trace:
        return out, res
    return out
